# revision 12
# baseline (speedup 1.0000x reference)
"""Trainium2 Bass kernel for CRF forward-algorithm loss (logsumexp scan).

Exp-domain matmul recurrence, split into two independent half-length chains
that run simultaneously and merge in the middle:

  forward :  ua_t = exp(emit_t - C) * (P  @ ua_{t-1}),  t = 0..255
  backward:  ub_{t-1} = P^T @ wb_t;  wb_{t-1} = exp(emit_{t-1} - C) * ub_{t-1},
             wb_511 = exp(emit_511 - C) * exp(trans[STOP]),  t = 511..256
  loss_b  =  log(sum_p ua_255[p] * ub_255[p]) - sum log zr + 512*C

v5: fp8 DoubleRow matmuls. Weights are e4m3, the recurrence state (ua/wb) is
e5m2; each PE instruction contracts K=256 (two 128-chunks packed on the
weight free axis), so a 512->512 matvec block is 8 matmuls instead of 16.
The per-round critical path is the PSUM-drain + sem + eviction + sem chain
(~900ns), hidden partially under the other chain's block.

Renorm every 8 rounds per chain (e5m2 range), off the critical path: z is
measured with two DoubleRow ones-matmuls, reciprocal zr is recorded and
folded into the emission tile of round r+2 (so the in-loop chain never
waits); host subtracts log zr.

Sharding: data-parallel over batch, 16 per core on 8 cores, host sums.
"""

import numpy as np
import ml_dtypes

import concourse.bass as bass
import concourse.bass_isa as bass_isa
import concourse.mybir as mybir
import concourse.tile as tile
from concourse import bacc
from concourse.bass_utils import run_bass_kernel_spmd

T = 512
S = 512
B = 128
NCORES = 8
BL = B // NCORES   # 16 per core
TC = 4
NP = 2             # chunk pairs (DoubleRow)
START = 510
STOP = 511
C = 7.0
RR = 8             # renorm period in rounds
DG = 8             # steps per DMA group
NR = S // 2        # rounds
RENF = list(range(5, 256, RR))            # fwd renorm rounds (32)
RENB = [r for r in RENF if r + 2 <= 254]  # bwd renorm rounds (31)
NRENT = len(RENF) + len(RENB)

F32 = mybir.dt.float32
BF16 = mybir.dt.bfloat16
FP8W = mybir.dt.float8e4
FP8U = mybir.dt.float8e5
DR = mybir.MatmulPerfMode.DoubleRow


def _build_program():
    nc = bacc.Bacc(
        "TRN2",
        target_bir_lowering=False,
        debug=False,
        enable_asserts=False,
        num_devices=NCORES,
    )

    ptf_d = nc.dram_tensor("ptf", [128, NP * TC * 2 * 128], FP8W,
                           kind="ExternalInput")
    ptb_d = nc.dram_tensor("ptb", [128, NP * TC * 2 * 128], FP8W,
                           kind="ExternalInput")
    u0_d = nc.dram_tensor("u0", [128, TC * BL], FP8U, kind="ExternalInput")
    ubstop_d = nc.dram_tensor("ubstop", [128, TC * BL], BF16,
                              kind="ExternalInput")
    em_d = nc.dram_tensor("emt", [S // DG, 128, DG * TC * BL], F32,
                          kind="ExternalInput")
    fin_d = nc.dram_tensor("fin", [1, BL], F32, kind="ExternalOutput")
    zs_d = nc.dram_tensor("zs", [1, NRENT * BL], F32, kind="ExternalOutput")

    with tile.TileContext(nc) as tc:
        with (
            tc.tile_pool(name="singles", bufs=1) as singles,
            tc.tile_pool(name="emfpool", bufs=3) as emfpool,
            tc.tile_pool(name="ehfpool", bufs=3) as ehfpool,
            tc.tile_pool(name="embpool", bufs=3) as embpool,
            tc.tile_pool(name="ehbpool", bufs=3) as ehbpool,
            tc.tile_pool(name="ufpool", bufs=2) as ufpool,
            tc.tile_pool(name="wbpool", bufs=2) as wbpool,
            tc.tile_pool(name="rnpool", bufs=2) as rnpool,
            tc.tile_pool(name="psfpool", bufs=2, space="PSUM") as psfpool,
            tc.tile_pool(name="psbpool", bufs=2, space="PSUM") as psbpool,
            tc.tile_pool(name="pzpool", bufs=2, space="PSUM") as pzpool,
        ):
            ptf = singles.tile([128, NP * TC * 2 * 128], FP8W)
            nc.sync.dma_start(out=ptf, in_=ptf_d[:, :])
            ptb = singles.tile([128, NP * TC * 2 * 128], FP8W)
            nc.sync.dma_start(out=ptb, in_=ptb_d[:, :])
            # [p, ip, j, two, m]
            ptfv = ptf.rearrange("p (ip j two m) -> p ip j two m",
                                 ip=NP, j=TC, two=2)
            ptbv = ptb.rearrange("p (ip j two m) -> p ip j two m",
                                 ip=NP, j=TC, two=2)
            uf = ufpool.tile([128, TC * BL], FP8U, name="uf", tag="uf")
            nc.sync.dma_start(out=uf, in_=u0_d[:, :])
            ubstop = singles.tile([128, TC * BL], BF16)
            nc.sync.dma_start(out=ubstop, in_=ubstop_d[:, :])
            negc_sb = singles.tile([128, 1], F32)
            nc.vector.memset(negc_sb, -C)
            ones_sb = singles.tile([128, 1], FP8W)
            nc.vector.memset(ones_sb, 1.0)
            zs_sb = singles.tile([1, NRENT * BL], F32)

            def load_group(gi, empool, ehpool, nm):
                em8 = empool.tile([128, DG * TC * BL], F32, name="em" + nm,
                                  tag="em" + nm)
                nc.sync.dma_start(out=em8, in_=em_d[gi])
                eh8 = ehpool.tile([128, DG * TC * BL], F32, name="eh" + nm,
                                  tag="eh" + nm)
                nc.scalar.activation(
                    eh8, em8, mybir.ActivationFunctionType.Exp,
                    bias=negc_sb, scale=1.0,
                )
                # [p, s, i, b]
                return eh8.rearrange("p (s i b) -> p s i b", s=DG, i=TC)

            ehf = load_group(0, emfpool, ehfpool, "f")
            ehb = load_group(S // DG - 1, embpool, ehbpool, "b")

            # wb_511 = exp(trans[STOP]) * exp(emit_511 - C)
            wb = wbpool.tile([128, TC * BL], FP8U, name="wb", tag="wb")
            nc.vector.tensor_mul(wb, ubstop, ehb[:, DG - 1, :, :])

            def dr_block(ps, ptv, u):
                uv = u.rearrange("p (i b) -> p i b", i=TC)
                for j in range(TC):
                    for ip in range(NP):
                        nc.tensor.matmul(
                            ps[:, j * BL:(j + 1) * BL],
                            ptv[:, ip, j, :, :],
                            uv[:, 2 * ip:2 * ip + 2, :],
                            start=(ip == 0), stop=(ip == NP - 1),
                            perf_mode=DR, skip_group_check=True,
                        )

            def renorm(u_new, eh_target, slot):
                """Measure z of u_new; record 1/z; fold 1/z into the s=7
                emission slice of eh_target (consumed two rounds later)."""
                uv = u_new.rearrange("p (i b) -> p i b", i=TC)
                zp = pzpool.tile([1, BL], F32, name="zp", tag="zp")
                for i in range(TC):
                    nc.tensor.matmul(
                        zp, ones_sb, uv[:, i, :],
                        start=(i == 0), stop=(i == TC - 1),
                        skip_group_check=True,
                    )
                zr = rnpool.tile([1, BL], F32, name="zr", tag="zr")
                nc.vector.reciprocal(zr, zp)
                nc.vector.tensor_copy(
                    zs_sb[0:1, slot * BL:(slot + 1) * BL], zr
                )
                zb = rnpool.tile([128, BL], F32, name="zb", tag="zb")
                nc.gpsimd.partition_broadcast(zb, zr)
                for i in range(TC):
                    nc.vector.tensor_mul(
                        eh_target[:, DG - 1, i, :],
                        eh_target[:, DG - 1, i, :], zb,
                    )

            slot = 0
            psb_last = None
            for r in range(NR):
                k, s = divmod(r, DG)
                if s == 0 and 1 <= k + 1 <= 31:
                    ehf_next = load_group(k + 1, emfpool, ehfpool, "f")
                if s == 0 and 32 <= 62 - k <= 62:
                    ehb_next = load_group(62 - k, embpool, ehbpool, "b")

                tb = 510 - r                     # bwd emission step
                kb, sb = divmod(tb, DG)

                psf = psfpool.tile([128, TC * BL], F32, name="psf", tag="psf")
                psb = psbpool.tile([128, TC * BL], F32, name="psb", tag="psb")
                dr_block(psf, ptfv, uf)
                dr_block(psb, ptbv, wb)

                uf_new = ufpool.tile([128, TC * BL], FP8U, name="uf", tag="uf")
                nc.vector.tensor_mul(uf_new, psf, ehf[:, s, :, :])
                if r < NR - 1:
                    wb_new = wbpool.tile([128, TC * BL], FP8U, name="wb",
                                         tag="wb")
                    nc.vector.tensor_mul(wb_new, psb, ehb[:, sb, :, :])
                else:
                    psb_last = psb

                if r in RENF:
                    renorm(uf_new, ehf, slot)
                    slot += 1
                    if r in RENB:
                        # bwd applies at round r+2 whose emission slice is
                        # s=7 of the *next* bwd tile (group transition at
                        # r % 8 == 6).
                        renorm(wb_new, ehb_next, slot)
                        slot += 1

                uf = uf_new
                if r < NR - 1:
                    wb = wb_new
                if s == DG - 1:
                    if 1 <= k + 1 <= 31:
                        ehf = ehf_next
                if sb == 0 and 32 <= kb - 1:
                    ehb = ehb_next

            # merge: fin_b = sum_p ua_255[p] * ub_255[p]
            ubf = singles.tile([128, TC * BL], F32)
            nc.vector.tensor_copy(ubf, psb_last)
            prod = singles.tile([128, TC * BL], F32)
            nc.vector.tensor_mul(prod, ubf, uf)
            m32 = singles.tile([128, 2 * BL], F32)
            nc.vector.tensor_add(m32, prod[:, 0:2 * BL], prod[:, 2 * BL:4 * BL])
            m16 = singles.tile([128, BL], F32)
            nc.vector.tensor_add(m16, m32[:, 0:BL], m32[:, BL:2 * BL])
            mall = singles.tile([128, BL], F32)
            nc.gpsimd.partition_all_reduce(
                mall, m16, 128, bass_isa.ReduceOp.add
            )
            fin_sb = singles.tile([1, BL], F32)
            nc.vector.tensor_copy(fin_sb, mall[0:1, :])
            nc.sync.dma_start(out=fin_d[0:1, :], in_=fin_sb)
            nc.sync.dma_start(out=zs_d[0:1, :], in_=zs_sb)

    nc.compile()
    return nc


def _chunk_dr(M):
    """[512, 512] -> [128, 2048]: weight pair (ip, j) holds K-chunks
    (2*ip, 2*ip+1) side by side: out[k, ((ip, j, two, m))] =
    M[(2*ip+two)*128 + k, j*128 + m]."""
    a = M.reshape(NP, 2, 128, TC, 128)       # [ip, two, k, j, m]
    return np.ascontiguousarray(a.transpose(2, 0, 3, 1, 4)).reshape(128, -1)


def _prep_inputs(emissions, transitions):
    e4 = ml_dtypes.float8_e4m3
    e5 = ml_dtypes.float8_e5m2
    bf = ml_dtypes.bfloat16
    P = np.exp(transitions.astype(np.float32))          # P[n, p]
    ptf_host = _chunk_dr(np.ascontiguousarray(P.T)).astype(e4)  # fwd lhsT=PT
    ptb_host = _chunk_dr(P).astype(e4)                           # bwd lhsT=P
    u0_host = np.zeros((128, TC * BL), dtype=e5)
    u0_host[START % 128, (START // 128) * BL:(START // 128 + 1) * BL] = 1.0
    pstop = np.exp(transitions[STOP].astype(np.float32))  # [p]
    ubstop_host = np.ascontiguousarray(
        np.repeat(pstop.reshape(TC, 128).T[:, :, None], BL, axis=2)
    ).reshape(128, TC * BL).astype(bf)

    in_maps = []
    for c in range(NCORES):
        sh = emissions[c * BL:(c + 1) * BL]             # [BL, S, T]
        a = sh.transpose(1, 2, 0)                       # [t, n, b]
        a = a.reshape(S // DG, DG, TC, 128, BL)         # [gi, s, i, k, b]
        emt = np.ascontiguousarray(a.transpose(0, 3, 1, 2, 4)).reshape(
            S // DG, 128, DG * TC * BL
        ).astype(np.float32)
        in_maps.append({"ptf": ptf_host, "ptb": ptb_host, "u0": u0_host,
                        "ubstop": ubstop_host, "emt": emt})
    return in_maps


def _loss_from_outputs(results):
    total = 0.0
    for res in results:
        fin = np.asarray(res["fin"], np.float64).reshape(BL)
        zs = np.asarray(res["zs"], np.float64).reshape(NRENT, BL)
        loss_b = np.log(fin) - np.log(zs).sum(axis=0) + S * C
        total += loss_b.sum()
    return np.float32(total)


def _run(inputs, **kwargs):
    emissions = np.asarray(inputs["inputs"], dtype=np.float32)
    transitions = np.asarray(inputs["transitions"], dtype=np.float32)
    assert emissions.shape == (B, S, T), emissions.shape
    nc = _build_program()
    in_maps = _prep_inputs(emissions, transitions)
    res = run_bass_kernel_spmd(nc, in_maps, core_ids=list(range(NCORES)), **kwargs)
    return _loss_from_outputs(res.results), res


def kernel(**inputs) -> np.ndarray:
    out, _ = _run(inputs)
    return out
